# revision 45
# baseline (speedup 1.0000x reference)
"""Trainium2 Bass kernel for single-head self-attention over image tokens.

Reference computation (per batch element b of 4):
    xf   = x[b] viewed as [N=4096 tokens, C=256]          (x stored [C, H*W] = xf.T)
    qkv  = xf @ w_qkv.T                                   -> q, k, v each [N, 512]
    sim  = (q * 64**-0.5) @ k.T                           [N, N]
    attn = softmax(sim, axis=-1)
    out  = (attn @ v) @ w_out.T + b_out + xf              [N, C]

Algebraic factorization (the key optimization): INNER=512 > C=256, so the
whole block collapses through two host-precomputed [256, 256] matrices
    M  = wq.T @ wk          (sim  = xf @ M @ xf.T, scaled at the exp)
    W2 = (w_out @ wv).T     (out  = attn @ (xf @ W2) + b + xf)
eliminating the q/k/v projections and the output projection entirely.
The host also precomputes t = xf @ M and xw = xf @ W2 in f32, so the
device only runs sim -> softmax -> attn @ xw: ~385 matmuls/core.

Sharding: 8 cores = 4 batches x 2 query-row halves (2048 rows each). Each
core's x is host-rotated so its query half is always columns 0:2048. No
collectives.

Precision (numpy-simulated 1.80e-2 vs the 2e-2 budget; sim has tracked hw
to ~1e-4 on every run):
  - sim contraction (256 dims): even 256-key pairs as one fp8e4 DoubleRow
    matmul (t and x quantized e4m3), odd pairs as two bf16 matmuls. Full
    fp8 would be 2.27e-2 -- over budget; half the keys costs half the
    noise power.
  - pT = exp(0.125*sim - 7): bf16 pairs via the ACT table engine into
    fp8e5; fp8 pairs via a Schraudolph affine-bit-pattern fast exp on the
    DVE (one tensor_scalar into the e5m2 bits through a uint8 bitcast,
    saturating) -- splits the exp load across two engines.
  - attn @ xw as fp8 DoubleRow over token-pair planes (xw in e4m3), f32
    PSUM accumulated across all 4096 keys; softmax denominator
    accumulated on the PE via DoubleRow all-ones colsum matmuls
    (broadcast to 128 partitions) -- the DVE/ACT engines have a ~2.3x
    silicon slowdown so no adder tree runs off-PE.
  - finalize is just out = po/l + b + x (bf16 out, f32 l from PSUM).

Schedule: ~5us of dependency-free bf16 warm-up matmuls hold the PE busy
through the input-DMA latency so the HAM clock gate reaches 2.4 GHz
before real work. One pass per 512-query slice over all 32 key chunks;
each slice's finalize (1/l, normalize, bias+residual, out DMA) is
deferred into the next slice's stream in four small chunks so the DVE
never bursts. The last slice finalizes in two 256-col halves with DMAs
spread across queues.
"""

import hashlib
import os
import shutil

import numpy as np

import concourse.bacc as bacc
import concourse.tile as tile
import concourse.mybir as mybir
from concourse.bass_utils import run_bass_kernel_spmd


def _install_neff_cache():
    """Disk-cache walrus NEFF compiles keyed on the BIR content hash.

    The axon PJRT path recompiles the NEFF in every fresh process (~minutes);
    the build here is deterministic, so identical BIR -> identical NEFF.
    """
    try:
        import concourse.bass2jax as bass2jax
        orig = bass2jax.compile_bir_kernel
        if getattr(orig, "_neff_cache_wrapped", False):
            return
        cache_dir = os.path.expanduser("~/.neuron-compile-cache/bass-neff")

        def cached(bir_json, tmpdir, neff_name="file.neff"):
            try:
                key = hashlib.sha256(
                    bir_json if isinstance(bir_json, bytes)
                    else bir_json.encode()).hexdigest()
                hit = os.path.join(cache_dir, key + ".neff")
                dst = os.path.join(tmpdir, neff_name)
                if os.path.exists(hit):
                    shutil.copyfile(hit, dst)
                    return dst
                neff = orig(bir_json, tmpdir, neff_name=neff_name)
                os.makedirs(cache_dir, exist_ok=True)
                tmp = hit + ".tmp%d" % os.getpid()
                shutil.copyfile(neff, tmp)
                os.replace(tmp, hit)
                return neff
            except Exception:
                return orig(bir_json, tmpdir, neff_name=neff_name)

        cached._neff_cache_wrapped = True
        bass2jax.compile_bir_kernel = cached
    except Exception:
        pass


_install_neff_cache()

F32 = mybir.dt.float32
BF16 = mybir.dt.bfloat16
F8E4 = mybir.dt.float8e4
F8E5 = mybir.dt.float8e5
U8 = mybir.dt.uint8
DR = mybir.MatmulPerfMode.DoubleRow
Exp = mybir.ActivationFunctionType.Exp
Add = mybir.AluOpType.add
Mult = mybir.AluOpType.mult
SHIFT = 7.0  # exp(scale*sim - SHIFT): keeps pT < e5m2 max; cancels in out
# Schraudolph fast-exp: e5m2 bits = round(4*log2(e)*(SCALE*sim - SHIFT) + 60
# - 4*C); C=0.0573 minimizes end-to-end rel err (numpy-swept)
EXPA = 4 * 0.125 * 1.4426950408889634
EXPB = 60 - 4 * 7 * 1.4426950408889634 - 4 * 0.0573

B = 4
C = 256          # model dim (2 chunks of 128)
N = 4096         # tokens per batch (64*64)
HALF = N // 2    # query rows per core
SCALE = 0.125    # 64 ** -0.5

NCORES = 8
NSL = 4          # query slices per core
SW = HALF // NSL # 512 query columns per slice
NJ8 = N // 128   # 32 key chunks
NPAIR = NJ8 // 2 # 16 key token-pairs (256 keys each)


def build_nc():
    nc = bacc.Bacc(None)
    x_r = nc.declare_dram_parameter("x_r", [C, N], BF16, isOutput=False)
    xq8_d = nc.declare_dram_parameter("xq8", [128, 2, N], F8E4,
                                      isOutput=False)
    tt_d = nc.declare_dram_parameter("tt", [C, HALF], BF16, isOutput=False)
    tp8_d = nc.declare_dram_parameter("tp8", [128, 2, HALF], F8E4,
                                      isOutput=False)
    xw_d = nc.declare_dram_parameter("xw", [128, NPAIR, 2, C], F8E4,
                                     isOutput=False)
    bout = nc.declare_dram_parameter("bout", [2, 128, 1], F32, isOutput=False)
    out = nc.declare_dram_parameter("out", [C, HALF], BF16, isOutput=True)

    mm = nc.tensor.matmul

    with tile.TileContext(nc) as tc:
        with tc.tile_pool(name="const", bufs=1) as const, \
             tc.tile_pool(name="work", bufs=2) as work, \
             tc.tile_pool(name="pp", bufs=1, space="PSUM") as pp:

            ones_sq_f = const.tile([128, 128], F32, tag="ones_sq_f",
                                   name="ones_sq_f")
            nc.vector.memset(ones_sq_f, 1.0)

            # ---- resident inputs, spread over the three DMA queues in
            # consumption order (each queue streams ~110GB/s serially, so
            # the slice-0 critical pieces must not share a queue)
            tt = [const.tile([128, HALF], BF16, tag=f"tt{cc}", name=f"tt{cc}")
                  for cc in range(2)]
            tp8 = const.tile([128, 2, HALF], F8E4, tag="tp8", name="tp8")
            xq8 = const.tile([128, 2, N], F8E4, tag="xq8", name="xq8")
            xr = [const.tile([128, N], BF16, tag=f"xr{cc}", name=f"xr{cc}")
                  for cc in range(2)]
            xwt = const.tile([128, NPAIR, 2, C], F8E4, tag="xwt", name="xwt")

            def xq8_piece(piece, eng):
                col = piece * (N // 4)
                eng.dma_start(xq8[:, :, col:col + N // 4],
                              xq8_d[:, :, col:col + N // 4])

            def xr_piece(cc, piece, eng):
                col = piece * (N // 4)
                eng.dma_start(xr[cc][:, col:col + N // 4],
                              x_r[cc * 128:(cc + 1) * 128, col:col + N // 4])

            def xw_piece(piece, eng):
                p0 = piece * (NPAIR // 4)
                eng.dma_start(xwt[:, p0:p0 + NPAIR // 4, :, :],
                              xw_d[:, p0:p0 + NPAIR // 4, :, :])

            # sync: xq8 head then tt chunk 0 then xq8 tail
            xq8_piece(0, nc.sync)
            nc.sync.dma_start(tt[0][:, 0:SW], tt_d[0:128, 0:SW])
            for piece in (1, 2, 3):
                xq8_piece(piece, nc.sync)
            # gpsimd: tp8 head, xr0 / xw even pieces
            nc.gpsimd.dma_start(tp8[:, :, 0:SW], tp8_d[:, :, 0:SW])
            xr_piece(0, 0, nc.gpsimd)
            xw_piece(0, nc.gpsimd)
            xr_piece(0, 1, nc.gpsimd)
            xw_piece(2, nc.gpsimd)
            xr_piece(0, 2, nc.gpsimd)
            xr_piece(0, 3, nc.gpsimd)
            # scalar: tt1 head, xr1 / xw odd pieces
            nc.scalar.dma_start(tt[1][:, 0:SW], tt_d[128:256, 0:SW])
            xr_piece(1, 0, nc.scalar)
            xw_piece(1, nc.scalar)
            xr_piece(1, 1, nc.scalar)
            xw_piece(3, nc.scalar)
            xr_piece(1, 2, nc.scalar)
            xr_piece(1, 3, nc.scalar)
            # remaining t slices + bias (late: needed from slice 1 on)
            for cc in range(2):
                nc.sync.dma_start(tt[cc][:, SW:],
                                  tt_d[cc * 128:(cc + 1) * 128, SW:])
            nc.sync.dma_start(tp8[:, :, SW:], tp8_d[:, :, SW:])
            bt = []
            for cc in range(2):
                t = const.tile([128, 1], F32, tag=f"b{cc}", name=f"b{cc}")
                nc.sync.dma_start(t, bout[cc])
                bt.append(t)

            # all-ones fp8 pair plane for the DoubleRow denominator colsum
            ones_psq = const.tile([128, 2, 128], F8E4, tag="ones_psq",
                                  name="ones_psq")
            nc.gpsimd.tensor_copy(ones_psq[:, 0, :], ones_sq_f)
            nc.gpsimd.tensor_copy(ones_psq[:, 1, :], ones_sq_f)
            nshift = const.tile([128, 1], F32, tag="nshift", name="nshift")
            nc.vector.memset(nshift, -SHIFT)

            # ---- PE warm-up: dependency-free bf16 matmuls bridge the
            # input-DMA latency and hold the HAM clock gate's busy window
            # so the PE reaches K=8/8 (2.4 GHz) as real work begins
            ones_bf = const.tile([128, 512], BF16, tag="ones_bf",
                                 name="ones_bf")
            ones_bf_f = const.tile([128, 512], F32, tag="ones_bf_f",
                                   name="ones_bf_f")
            nc.vector.memset(ones_bf_f, 1.0)
            nc.vector.tensor_copy(ones_bf, ones_bf_f)
            warm_ps = pp.tile([128, 512], F32, tag="aux", bufs=2,
                              name="warm_ps")
            NWARM = 22
            for i in range(NWARM):
                mm(warm_ps, ones_bf[:, 0:128], ones_bf,
                   start=(i == 0), stop=(i == NWARM - 1))
            # pre-trigger the ACT exp table load (~2.7us) during the
            # input DMA wait so the first real exp doesn't pay it
            warm_act = const.tile([128, 1], F32, tag="warm_act",
                                  name="warm_act")
            nc.scalar.activation(warm_act, ones_sq_f[:, 0:1], Exp,
                                 scale=1.0)

            # ---- attention: one pass per query slice over all keys ----
            deferred = {}   # j8 -> [fns]: prev slice finalize chunks
            for s in range(NSL):
                sl = slice(s * SW, (s + 1) * SW)
                tail = s == NSL - 1
                po = [pp.tile([128, SW], F32, tag=f"po{cc}", bufs=2,
                              name=f"po{cc}") for cc in range(2)]
                # softmax denominator: accumulated on the PE via DoubleRow
                # all-ones colsum matmuls (broadcast to all 128 partitions)
                pb = pp.tile([128, SW], F32, tag="aux", bufs=2, name="pb")
                ptp = []

                def drain_pair(p, po=po, pb=pb):
                    # attn@xw for token-pair p: fp8 DoubleRow, 256 keys
                    # contracted per mm, accumulating over all 16 pairs
                    for cc in range(2):
                        mm(po[cc], xwt[:, p, :, cc * 128:(cc + 1) * 128],
                           ptp[p][:, :, :],
                           start=(p == 0), stop=(p == NPAIR - 1),
                           perf_mode=DR)
                    mm(pb, ones_psq, ptp[p][:, :, :],
                       start=(p == 0), stop=(p == NPAIR - 1),
                       perf_mode=DR)

                for j8 in range(NJ8):
                    ps = pp.tile([128, SW], F32, tag="sim", bufs=2,
                                 name="ps_s")
                    # even pairs fp8/DVE, odd bf16/ACT; the last two pairs
                    # swap so the slice hand-off exp rides the lighter DVE
                    pp_ = j8 // 2
                    fp8_pair = (pp_ % 2 == 0) if pp_ < 14 else (pp_ == 15)
                    if fp8_pair:
                        # even key pairs: fp8 DoubleRow, 256-dim contraction
                        # in one mm (rel-err budget allows half the keys)
                        mm(ps, xq8[:, :, j8 * 128:(j8 + 1) * 128],
                           tp8[:, :, sl], start=True, stop=True,
                           perf_mode=DR)
                    else:
                        for cc in range(2):
                            mm(ps, xr[cc][:, j8 * 128:(j8 + 1) * 128],
                               tt[cc][:, sl], start=(cc == 0), stop=(cc == 1))
                    p, parity = divmod(j8, 2)
                    if parity == 0:
                        t = work.tile([128, 2, SW], F8E5, tag="pt", bufs=4,
                                      name="pt")
                        ptp.append(t)
                    if fp8_pair:
                        # Schraudolph fast-exp on the DVE: affine map into
                        # the e5m2 bit pattern (uint8 convert saturates);
                        # splits the exp load across ACT and DVE
                        nc.vector.tensor_scalar(
                            ptp[p][:, parity, :].bitcast(U8), ps,
                            EXPA, EXPB, op0=Mult, op1=Add)
                    else:
                        nc.scalar.activation(ptp[p][:, parity, :], ps, Exp,
                                             scale=SCALE, bias=nshift)
                    if parity == 1 and p > 0:
                        drain_pair(p - 1)
                    for fn in deferred.pop(j8, ()):
                        fn()
                drain_pair(NPAIR - 1)

                # ---- finalize slice s: out = po/l + b + x, in chunks ----
                def make_fin(off, wdt, s=s, po=po, pb=pb, dma_engs=None):
                    state = {}

                    def f_bc():
                        bc = work.tile([128, wdt], F32, tag="bc", bufs=2,
                                       name="bc")
                        rsc = work.tile([128, wdt], F32, tag="rsc", bufs=2,
                                        name="rsc")
                        nc.vector.reciprocal_approx_accurate(
                            bc, pb[:, off:off + wdt], rsc)
                        state["bc"] = bc

                    def f_cc(cc):
                        def fn():
                            sl2 = slice(s * SW + off, s * SW + off + wdt)
                            fo = work.tile([128, wdt], F32, tag="fo", bufs=2,
                                           name="fo")
                            nc.vector.tensor_mul(fo, po[cc][:, off:off + wdt],
                                                 state["bc"])
                            fo2 = work.tile([128, wdt], BF16, tag="fo2",
                                            bufs=2, name="fo2")
                            nc.vector.scalar_tensor_tensor(
                                fo2, xr[cc][:, sl2], bt[cc], fo,
                                op0=Add, op1=Add)
                            deng = dma_engs[cc] if dma_engs else nc.sync
                            deng.dma_start(out[cc * 128:(cc + 1) * 128, sl2],
                                           fo2)
                        return fn
                    return f_bc, f_cc(0), f_cc(1)

                if not tail:
                    fb, f0, f1 = make_fin(0, SW)
                    deferred = {2: [fb], 8: [f0], 16: [f1]}
                else:
                    # kernel tail: two 256-col halves so the first half's
                    # normalize/output overlaps the second's
                    hwd = SW // 2
                    b0, c0, d0 = make_fin(0, hwd, dma_engs=(nc.scalar,
                                                            nc.sync))
                    b1, c1, d1 = make_fin(hwd, hwd, dma_engs=(nc.gpsimd,
                                                              nc.scalar))
                    b0(); b1(); c0(); d0(); c1(); d1()

    nc.finalize()
    return nc


_NC_CACHE = None


def _get_nc():
    global _NC_CACHE
    if _NC_CACHE is None:
        _NC_CACHE = build_nc()
    return _NC_CACHE


def prepare_in_maps(x, w_qkv, w_out, b_out):
    x = np.asarray(x, dtype=np.float32)
    w_qkv = np.asarray(w_qkv, dtype=np.float32)
    w_out = np.asarray(w_out, dtype=np.float32)
    b_out = np.asarray(b_out, dtype=np.float32)

    import ml_dtypes
    bf16 = ml_dtypes.bfloat16
    f8e4 = ml_dtypes.float8_e4m3
    wq, wk, wv = w_qkv[:512], w_qkv[512:1024], w_qkv[1024:]
    M = (wq.T.astype(np.float64) @ wk.astype(np.float64)).astype(np.float32)
    W2 = (w_out.astype(np.float64) @ wv.astype(np.float64)).T.astype(np.float32)
    bout = np.ascontiguousarray(b_out.reshape(2, 128, 1))

    xr = x.reshape(B, C, N)
    in_maps = []
    for c in range(NCORES):
        b, h = divmod(c, 2)
        if h == 0:
            x_rot = xr[b]
        else:  # rotate so this core's query half sits in columns 0:HALF
            x_rot = np.concatenate([xr[b][:, HALF:], xr[b][:, :HALF]], axis=1)
        x_bf = x_rot.astype(bf16)
        xf = np.ascontiguousarray(x_rot.T)                       # [N, C] f32
        # t = xf @ M for the query half, stored transposed [C, HALF]
        tT = np.ascontiguousarray((xf[:HALF] @ M).T)             # [256, HALF]
        tp8 = np.ascontiguousarray(
            tT.astype(f8e4).reshape(2, 128, HALF).transpose(1, 0, 2))
        # c-pair planes: xq8[p, r, j] = fp8(x_rot[r*128+p, j])
        xq8 = np.ascontiguousarray(
            x_bf.astype(f8e4).reshape(2, 128, N).transpose(1, 0, 2))
        # xw = xf @ W2 token-pair planes:
        # xw[p, pair, plane, c] = fp8(xw[(pair*2+plane)*128+p, c])
        xw8 = (xf @ W2).astype(f8e4)                             # [N, C]
        xw = np.ascontiguousarray(
            xw8.reshape(NPAIR, 2, 128, C).transpose(2, 0, 1, 3))
        in_maps.append({
            "x_r": x_bf,
            "xq8": xq8,
            "tt": tT.astype(bf16),
            "tp8": tp8,
            "xw": xw,
            "bout": bout,
        })
    return in_maps


def postprocess(results):
    outs = [results[c]["out"] for c in range(NCORES)]
    full = np.stack([np.concatenate([outs[2 * b], outs[2 * b + 1]], axis=1)
                     for b in range(B)])               # [B, C, N]
    return full.reshape(B, C, 64, 64).astype(np.float32)


def kernel(x, w_qkv, w_out, b_out):
    in_maps = prepare_in_maps(x, w_qkv, w_out, b_out)
    res = run_bass_kernel_spmd(_get_nc(), in_maps, core_ids=list(range(NCORES)))
    return postprocess(res.results)


# revision 50
# speedup vs baseline: 1.0326x; 1.0326x over previous
"""Trainium2 Bass kernel for single-head self-attention over image tokens.

Reference computation (per batch element b of 4):
    xf   = x[b] viewed as [N=4096 tokens, C=256]          (x stored [C, H*W] = xf.T)
    qkv  = xf @ w_qkv.T                                   -> q, k, v each [N, 512]
    sim  = (q * 64**-0.5) @ k.T                           [N, N]
    attn = softmax(sim, axis=-1)
    out  = (attn @ v) @ w_out.T + b_out + xf              [N, C]

Algebraic factorization (the key optimization): INNER=512 > C=256, so the
whole block collapses through two host-precomputed [256, 256] matrices
    M  = wq.T @ wk          (sim  = xf @ M @ xf.T, scaled at the exp)
    W2 = (w_out @ wv).T     (out  = attn @ (xf @ W2) + b + xf)
eliminating the q/k/v projections and the output projection entirely.
The host also precomputes t = xf @ M and xw = xf @ W2 in f32, so the
device only runs sim -> softmax -> attn @ xw: ~385 matmuls/core.

Sharding: 8 cores = 4 batches x 2 query-row halves (2048 rows each). Each
core's x is host-rotated so its query half is always columns 0:2048. No
collectives.

Precision (numpy-simulated 1.80e-2 vs the 2e-2 budget; sim has tracked hw
to ~1e-4 on every run):
  - sim contraction (256 dims): even 256-key pairs as one fp8e4 DoubleRow
    matmul (t and x quantized e4m3), odd pairs as two bf16 matmuls. Full
    fp8 would be 2.27e-2 -- over budget; half the keys costs half the
    noise power.
  - pT = exp(0.125*sim - 7): bf16 pairs via the ACT table engine into
    fp8e5; fp8 pairs via a Schraudolph affine-bit-pattern fast exp on the
    DVE (one tensor_scalar into the e5m2 bits through a uint8 bitcast,
    saturating) -- splits the exp load across two engines.
  - attn @ xw as fp8 DoubleRow over token-pair planes (xw in e4m3), f32
    PSUM accumulated across all 4096 keys; softmax denominator
    accumulated on the PE via DoubleRow all-ones colsum matmuls
    (broadcast to 128 partitions) -- the DVE/ACT engines have a ~2.3x
    silicon slowdown so no adder tree runs off-PE.
  - finalize is just out = po/l + b + x (bf16 out, f32 l from PSUM).

Schedule: ~5us of dependency-free bf16 warm-up matmuls hold the PE busy
through the input-DMA latency so the HAM clock gate reaches 2.4 GHz
before real work. One pass per 512-query slice over all 32 key chunks;
each slice's finalize (1/l, normalize, bias+residual, out DMA) is
deferred into the next slice's stream in four small chunks so the DVE
never bursts. The last slice finalizes in two 256-col halves with DMAs
spread across queues.
"""

import hashlib
import os
import shutil

import numpy as np

import concourse.bacc as bacc
import concourse.tile as tile
import concourse.mybir as mybir
from concourse.bass_utils import run_bass_kernel_spmd


def _install_neff_cache():
    """Disk-cache walrus NEFF compiles keyed on the BIR content hash.

    The axon PJRT path recompiles the NEFF in every fresh process (~minutes);
    the build here is deterministic, so identical BIR -> identical NEFF.
    """
    try:
        import concourse.bass2jax as bass2jax
        orig = bass2jax.compile_bir_kernel
        if getattr(orig, "_neff_cache_wrapped", False):
            return
        cache_dir = os.path.expanduser("~/.neuron-compile-cache/bass-neff")

        def cached(bir_json, tmpdir, neff_name="file.neff"):
            try:
                key = hashlib.sha256(
                    bir_json if isinstance(bir_json, bytes)
                    else bir_json.encode()).hexdigest()
                hit = os.path.join(cache_dir, key + ".neff")
                dst = os.path.join(tmpdir, neff_name)
                if os.path.exists(hit):
                    shutil.copyfile(hit, dst)
                    return dst
                neff = orig(bir_json, tmpdir, neff_name=neff_name)
                os.makedirs(cache_dir, exist_ok=True)
                tmp = hit + ".tmp%d" % os.getpid()
                shutil.copyfile(neff, tmp)
                os.replace(tmp, hit)
                return neff
            except Exception:
                return orig(bir_json, tmpdir, neff_name=neff_name)

        cached._neff_cache_wrapped = True
        bass2jax.compile_bir_kernel = cached
    except Exception:
        pass


_install_neff_cache()

F32 = mybir.dt.float32
BF16 = mybir.dt.bfloat16
F8E4 = mybir.dt.float8e4
F8E5 = mybir.dt.float8e5
U8 = mybir.dt.uint8
DR = mybir.MatmulPerfMode.DoubleRow
Exp = mybir.ActivationFunctionType.Exp
Add = mybir.AluOpType.add
Mult = mybir.AluOpType.mult
SHIFT = 7.0  # exp(scale*sim - SHIFT): keeps pT < e5m2 max; cancels in out
# Schraudolph fast-exp: e5m2 bits = round(4*log2(e)*(SCALE*sim - SHIFT) + 60
# - 4*C); C=0.0573 minimizes end-to-end rel err (numpy-swept)
EXPA = 4 * 0.125 * 1.4426950408889634
EXPB = 60 - 4 * 7 * 1.4426950408889634 - 4 * 0.0573

B = 4
C = 256          # model dim (2 chunks of 128)
N = 4096         # tokens per batch (64*64)
HALF = N // 2    # query rows per core
SCALE = 0.125    # 64 ** -0.5

NCORES = 8
NSL = 4          # query slices per core
SW = HALF // NSL # 512 query columns per slice
NJ8 = N // 128   # 32 key chunks
NPAIR = NJ8 // 2 # 16 key token-pairs (256 keys each)


def build_nc():
    nc = bacc.Bacc(None)
    x_r = nc.declare_dram_parameter("x_r", [C, N], BF16, isOutput=False)
    xq8_d = nc.declare_dram_parameter("xq8", [128, 2, N], F8E4,
                                      isOutput=False)
    tt_d = nc.declare_dram_parameter("tt", [C, HALF], BF16, isOutput=False)
    tp8_d = nc.declare_dram_parameter("tp8", [128, 2, HALF], F8E4,
                                      isOutput=False)
    xw_d = nc.declare_dram_parameter("xw", [128, NPAIR, 2, C], F8E4,
                                     isOutput=False)
    bout = nc.declare_dram_parameter("bout", [2, 128, 1], F32, isOutput=False)
    out = nc.declare_dram_parameter("out", [C, HALF], BF16, isOutput=True)

    mm = nc.tensor.matmul

    with tile.TileContext(nc) as tc:
        with tc.tile_pool(name="const", bufs=1) as const, \
             tc.tile_pool(name="work", bufs=2) as work, \
             tc.tile_pool(name="pp", bufs=1, space="PSUM") as pp:

            ones_sq_f = const.tile([128, 128], F32, tag="ones_sq_f",
                                   name="ones_sq_f")
            nc.vector.memset(ones_sq_f, 1.0)

            # ---- resident inputs, spread over the three DMA queues in
            # consumption order (each queue streams ~110GB/s serially, so
            # the slice-0 critical pieces must not share a queue)
            tt = [const.tile([128, HALF], BF16, tag=f"tt{cc}", name=f"tt{cc}")
                  for cc in range(2)]
            tp8 = const.tile([128, 2, HALF], F8E4, tag="tp8", name="tp8")
            xq8 = const.tile([128, 2, N], F8E4, tag="xq8", name="xq8")
            xr = [const.tile([128, N], BF16, tag=f"xr{cc}", name=f"xr{cc}")
                  for cc in range(2)]
            xwt = const.tile([128, NPAIR, 2, C], F8E4, tag="xwt", name="xwt")

            def xq8_piece(piece, eng):
                col = piece * (N // 4)
                eng.dma_start(xq8[:, :, col:col + N // 4],
                              xq8_d[:, :, col:col + N // 4])

            def xr_piece(cc, piece, eng):
                col = piece * (N // 4)
                eng.dma_start(xr[cc][:, col:col + N // 4],
                              x_r[cc * 128:(cc + 1) * 128, col:col + N // 4])

            def xw_piece(piece, eng):
                p0 = piece * (NPAIR // 4)
                eng.dma_start(xwt[:, p0:p0 + NPAIR // 4, :, :],
                              xw_d[:, p0:p0 + NPAIR // 4, :, :])

            # sync: xq8 with a small leading piece, then late-need tails
            for (col, w) in ((0, 512), (512, 1024), (1536, 1024),
                             (2560, 1536)):
                nc.sync.dma_start(xq8[:, :, col:col + w],
                                  xq8_d[:, :, col:col + w])
            # gpsimd: tp8 head, xr0 / xw even pieces
            nc.gpsimd.dma_start(tp8[:, :, 0:SW], tp8_d[:, :, 0:SW])
            xr_piece(0, 0, nc.gpsimd)
            xw_piece(0, nc.gpsimd)
            xr_piece(0, 1, nc.gpsimd)
            xw_piece(2, nc.gpsimd)
            xr_piece(0, 2, nc.gpsimd)
            xr_piece(0, 3, nc.gpsimd)
            # scalar: tt heads, xr1 / xw odd pieces
            nc.scalar.dma_start(tt[0][:, 0:SW], tt_d[0:128, 0:SW])
            nc.scalar.dma_start(tt[1][:, 0:SW], tt_d[128:256, 0:SW])
            xr_piece(1, 0, nc.scalar)
            xw_piece(1, nc.scalar)
            xr_piece(1, 1, nc.scalar)
            xw_piece(3, nc.scalar)
            xr_piece(1, 2, nc.scalar)
            xr_piece(1, 3, nc.scalar)
            # remaining t slices + bias (late: needed from slice 1 on)
            for cc in range(2):
                nc.sync.dma_start(tt[cc][:, SW:],
                                  tt_d[cc * 128:(cc + 1) * 128, SW:])
            nc.sync.dma_start(tp8[:, :, SW:], tp8_d[:, :, SW:])
            bt = []
            for cc in range(2):
                t = const.tile([128, 1], F32, tag=f"b{cc}", name=f"b{cc}")
                nc.sync.dma_start(t, bout[cc])
                bt.append(t)

            # all-ones fp8 pair plane for the DoubleRow denominator colsum
            ones_psq = const.tile([128, 2, 128], F8E4, tag="ones_psq",
                                  name="ones_psq")
            nc.gpsimd.tensor_copy(ones_psq[:, 0, :], ones_sq_f)
            nc.gpsimd.tensor_copy(ones_psq[:, 1, :], ones_sq_f)
            nshift = const.tile([128, 1], F32, tag="nshift", name="nshift")
            nc.vector.memset(nshift, -SHIFT)

            # ---- PE warm-up: dependency-free bf16 matmuls bridge the
            # input-DMA latency and hold the HAM clock gate's busy window
            # so the PE reaches K=8/8 (2.4 GHz) as real work begins
            ones_bf = const.tile([128, 512], BF16, tag="ones_bf",
                                 name="ones_bf")
            ones_bf_f = const.tile([128, 512], F32, tag="ones_bf_f",
                                   name="ones_bf_f")
            nc.vector.memset(ones_bf_f, 1.0)
            nc.vector.tensor_copy(ones_bf, ones_bf_f)
            warm_ps = pp.tile([128, 512], F32, tag="aux", bufs=2,
                              name="warm_ps")
            NWARM = 15
            for i in range(NWARM):
                mm(warm_ps, ones_bf[:, 0:128], ones_bf,
                   start=(i == 0), stop=(i == NWARM - 1))
            # pre-trigger the ACT exp table load (~2.7us) during the
            # input DMA wait so the first real exp doesn't pay it
            warm_act = const.tile([128, 1], F32, tag="warm_act",
                                  name="warm_act")
            nc.scalar.activation(warm_act, ones_sq_f[:, 0:1], Exp,
                                 scale=1.0)

            # ---- attention: one pass per query slice over all keys ----
            deferred = {}   # j8 -> [fns]: prev slice finalize chunks
            for s in range(NSL):
                sl = slice(s * SW, (s + 1) * SW)
                tail = s == NSL - 1
                po = [pp.tile([128, SW], F32, tag=f"po{cc}", bufs=2,
                              name=f"po{cc}") for cc in range(2)]
                # softmax denominator: accumulated on the PE via DoubleRow
                # all-ones colsum matmuls (broadcast to all 128 partitions)
                pb = pp.tile([128, SW], F32, tag="aux", bufs=2, name="pb")
                ptp = []

                def drain_pair(p, po=po, pb=pb):
                    # attn@xw for token-pair p: fp8 DoubleRow, 256 keys
                    # contracted per mm, accumulating over all 16 pairs
                    for cc in range(2):
                        mm(po[cc], xwt[:, p, :, cc * 128:(cc + 1) * 128],
                           ptp[p][:, :, :],
                           start=(p == 0), stop=(p == NPAIR - 1),
                           perf_mode=DR)
                    mm(pb, ones_psq, ptp[p][:, :, :],
                       start=(p == 0), stop=(p == NPAIR - 1),
                       perf_mode=DR)

                for j8 in range(NJ8):
                    ps = pp.tile([128, SW], F32, tag="sim", bufs=2,
                                 name="ps_s")
                    fp8_pair = (j8 // 2) % 2 == 0
                    if fp8_pair:
                        # even key pairs: fp8 DoubleRow, 256-dim contraction
                        # in one mm (rel-err budget allows half the keys)
                        mm(ps, xq8[:, :, j8 * 128:(j8 + 1) * 128],
                           tp8[:, :, sl], start=True, stop=True,
                           perf_mode=DR)
                    else:
                        for cc in range(2):
                            mm(ps, xr[cc][:, j8 * 128:(j8 + 1) * 128],
                               tt[cc][:, sl], start=(cc == 0), stop=(cc == 1))
                    p, parity = divmod(j8, 2)
                    if parity == 0:
                        t = work.tile([128, 2, SW], F8E5, tag="pt", bufs=5,
                                      name="pt")
                        ptp.append(t)
                    if fp8_pair:
                        # Schraudolph fast-exp on the DVE: affine map into
                        # the e5m2 bit pattern (uint8 convert saturates);
                        # splits the exp load across ACT and DVE
                        nc.vector.tensor_scalar(
                            ptp[p][:, parity, :].bitcast(U8), ps,
                            EXPA, EXPB, op0=Mult, op1=Add)
                    else:
                        nc.scalar.activation(ptp[p][:, parity, :], ps, Exp,
                                             scale=SCALE, bias=nshift)
                    # drain two pairs behind the exp so the first drain's
                    # xw stationary has extra DMA latency slack
                    if parity == 1 and p > 1:
                        drain_pair(p - 2)
                    for fn in deferred.pop(j8, ()):
                        fn()
                drain_pair(NPAIR - 2)
                drain_pair(NPAIR - 1)

                # ---- finalize slice s: out = po/l + b + x, in chunks ----
                def make_fin(off, wdt, s=s, po=po, pb=pb, dma_engs=None):
                    state = {}

                    def f_bc():
                        bc = work.tile([128, wdt], F32, tag="bc", bufs=2,
                                       name="bc")
                        rsc = work.tile([128, wdt], F32, tag="rsc", bufs=2,
                                        name="rsc")
                        nc.vector.reciprocal_approx_accurate(
                            bc, pb[:, off:off + wdt], rsc)
                        state["bc"] = bc

                    def f_cc(cc):
                        def fn():
                            sl2 = slice(s * SW + off, s * SW + off + wdt)
                            fo = work.tile([128, wdt], F32, tag="fo", bufs=2,
                                           name="fo")
                            nc.vector.tensor_mul(fo, po[cc][:, off:off + wdt],
                                                 state["bc"])
                            fo2 = work.tile([128, wdt], BF16, tag="fo2",
                                            bufs=2, name="fo2")
                            nc.vector.scalar_tensor_tensor(
                                fo2, xr[cc][:, sl2], bt[cc], fo,
                                op0=Add, op1=Add)
                            deng = dma_engs[cc] if dma_engs else nc.sync
                            deng.dma_start(out[cc * 128:(cc + 1) * 128, sl2],
                                           fo2)
                        return fn
                    return f_bc, f_cc(0), f_cc(1)

                if not tail:
                    fb, f0, f1 = make_fin(0, SW)
                    deferred = {2: [fb], 8: [f0], 16: [f1]}
                else:
                    # kernel tail: two 256-col halves so the first half's
                    # normalize/output overlaps the second's
                    hwd = SW // 2
                    b0, c0, d0 = make_fin(0, hwd, dma_engs=(nc.scalar,
                                                            nc.sync))
                    b1, c1, d1 = make_fin(hwd, hwd, dma_engs=(nc.gpsimd,
                                                              nc.scalar))
                    b0(); b1(); c0(); d0(); c1(); d1()

    nc.finalize()
    return nc


_NC_CACHE = None


def _get_nc():
    global _NC_CACHE
    if _NC_CACHE is None:
        _NC_CACHE = build_nc()
    return _NC_CACHE


def prepare_in_maps(x, w_qkv, w_out, b_out):
    x = np.asarray(x, dtype=np.float32)
    w_qkv = np.asarray(w_qkv, dtype=np.float32)
    w_out = np.asarray(w_out, dtype=np.float32)
    b_out = np.asarray(b_out, dtype=np.float32)

    import ml_dtypes
    bf16 = ml_dtypes.bfloat16
    f8e4 = ml_dtypes.float8_e4m3
    wq, wk, wv = w_qkv[:512], w_qkv[512:1024], w_qkv[1024:]
    M = (wq.T.astype(np.float64) @ wk.astype(np.float64)).astype(np.float32)
    W2 = (w_out.astype(np.float64) @ wv.astype(np.float64)).T.astype(np.float32)
    bout = np.ascontiguousarray(b_out.reshape(2, 128, 1))

    xr = x.reshape(B, C, N)
    in_maps = []
    for c in range(NCORES):
        b, h = divmod(c, 2)
        if h == 0:
            x_rot = xr[b]
        else:  # rotate so this core's query half sits in columns 0:HALF
            x_rot = np.concatenate([xr[b][:, HALF:], xr[b][:, :HALF]], axis=1)
        x_bf = x_rot.astype(bf16)
        xf = np.ascontiguousarray(x_rot.T)                       # [N, C] f32
        # t = xf @ M for the query half, stored transposed [C, HALF]
        tT = np.ascontiguousarray((xf[:HALF] @ M).T)             # [256, HALF]
        tp8 = np.ascontiguousarray(
            tT.astype(f8e4).reshape(2, 128, HALF).transpose(1, 0, 2))
        # c-pair planes: xq8[p, r, j] = fp8(x_rot[r*128+p, j])
        xq8 = np.ascontiguousarray(
            x_bf.astype(f8e4).reshape(2, 128, N).transpose(1, 0, 2))
        # xw = xf @ W2 token-pair planes:
        # xw[p, pair, plane, c] = fp8(xw[(pair*2+plane)*128+p, c])
        xw8 = (xf @ W2).astype(f8e4)                             # [N, C]
        xw = np.ascontiguousarray(
            xw8.reshape(NPAIR, 2, 128, C).transpose(2, 0, 1, 3))
        in_maps.append({
            "x_r": x_bf,
            "xq8": xq8,
            "tt": tT.astype(bf16),
            "tp8": tp8,
            "xw": xw,
            "bout": bout,
        })
    return in_maps


def postprocess(results):
    outs = [results[c]["out"] for c in range(NCORES)]
    full = np.stack([np.concatenate([outs[2 * b], outs[2 * b + 1]], axis=1)
                     for b in range(B)])               # [B, C, N]
    return full.reshape(B, C, 64, 64).astype(np.float32)


def kernel(x, w_qkv, w_out, b_out):
    in_maps = prepare_in_maps(x, w_qkv, w_out, b_out)
    res = run_bass_kernel_spmd(_get_nc(), in_maps, core_ids=list(range(NCORES)))
    return postprocess(res.results)
